# revision 18
# baseline (speedup 1.0000x reference)
"""MHSA3D Trainium2 kernel: 8-way head-parallel flash-style attention.

Problem (hardcoded): B=1, C=128, D=H=W=16 -> N=4096 tokens, 8 heads,
dh=16, dv=128.  Each of the 8 NeuronCores computes one head end-to-end:
qkv projection (its head's slice), S^T = k'^T q' logits in [j, i]
layout, exp on ScalarE (no max subtraction -- fp32 exp cannot overflow
for this data), PV accumulation with an appended ones-row producing the
softmax denominator, then normalize.

Perf-critical choices:
- The qk contraction is only dh=16, but K=16 matmuls never trip the PE
  HAM activity monitor, so the PE clock stays throttled at 1.2 GHz.
  We zero-pad the contraction to K=128 (stationary k' rows 16-127 are
  exact zeros); matmul time scales with N only, and full-K matmuls keep
  the PE warm at 2.4 GHz.
- q'/k' in fp16 (logits reach +-46, but q/k values are small; fp16's
  extra mantissa bits halve the logit rounding error vs bf16), P and v
  in bf16 (P reaches exp(40), beyond fp16 range).
- PV for group g is emitted after qk for group g+1 so the PE FIFO never
  head-of-line blocks on the exp; the per-eighth normalize tail is
  deferred into the next eighth the same way.

Host side: fold the 1/sqrt(dh) scale into wq/bq, fold (b_k + positional
embedding) into a [16, N] bias plane, slice per-head weights, run the
SPMD program on cores 0-7, and concatenate the per-head [16, N] outputs.
"""

import numpy as np

NHEADS = 8
DV = 128
DH = DV // NHEADS  # 16
C = 128
N = 4096
ECOLS = 512        # i-columns handled per output tile ("eighth")
NE = N // ECOLS    # 8
JW = 128           # keys per j-block
NJB = N // JW      # 32
NG = NJB // 2      # 16 j-block pairs per eighth

_compiled = None


def _build_program():
    import concourse.bacc as bacc
    import concourse.mybir as mybir
    import concourse.tile as tile

    f32 = mybir.dt.float32
    bf16 = mybir.dt.bfloat16
    fp16 = mybir.dt.float16
    EXP = mybir.ActivationFunctionType.Exp
    ADD = mybir.AluOpType.add
    MULT = mybir.AluOpType.mult

    nc = bacc.Bacc("TRN2", target_bir_lowering=False, debug=False,
                   num_devices=NHEADS)

    x_d = nc.dram_tensor("x", [C, N], fp16, kind="ExternalInput")
    # w cols: 0-15 wq*scale, 16-31 wk, 32-47 wv
    w_d = nc.dram_tensor("w", [C, 48], fp16, kind="ExternalInput")
    # bias plane rows: 0-15 bq*scale (bcast), 16-31 bk+emb
    b_d = nc.dram_tensor("bias", [32, N], f32, kind="ExternalInput")
    bv_d = nc.dram_tensor("bv", [DH, 1], f32, kind="ExternalInput")
    z_d = nc.dram_tensor("zeros", [C, N], fp16, kind="ExternalInput")
    o_d = nc.dram_tensor("out", [DH, N], f32, kind="ExternalOutput")

    with tile.TileContext(nc) as tc:
        with (
            tc.tile_pool(name="const", bufs=1) as const,
            tc.tile_pool(name="pt", bufs=4) as ptp,
            tc.tile_pool(name="o", bufs=2) as op,
            tc.tile_pool(name="st", bufs=3, space="PSUM") as stp,
            tc.tile_pool(name="acc", bufs=2, space="PSUM") as accp,
        ):
            x_s = const.tile([C, N], fp16)
            w_s = const.tile([C, 48], fp16)
            biasf = const.tile([32, N], f32)
            bv_s = const.tile([DH, 1], f32)
            # qz rows: 0-15 q'; 16-31 k' (junk for the matmul); 32-127 zero
            qz = const.tile([128, N], fp16)
            # kz rows: 0-15 k'; 16-127 exact zero (the K=128 zero-pad)
            kz = const.tile([128, N], fp16)
            vaugT = const.tile([128, 33 * NJB], bf16)  # [j, (v^T |0*16| 1)]
            ones16 = const.tile([1, DH], f32)
            zerob = const.tile([128, 1], f32)
            scratch1 = const.tile([128, 1], f32)

            # Zeros land first: Tile tracks WAW at tile granularity, so
            # the bias-add into qz[0:32] serializes behind the qz zero-fill.
            for q2 in range(2):
                q2s = slice(q2 * 2048, (q2 + 1) * 2048)
                nc.gpsimd.dma_start(qz[32:128, q2s], z_d.ap()[0:96, q2s])
            for q4 in range(4):
                q4s = slice(q4 * 1024, (q4 + 1) * 1024)
                nc.sync.dma_start(x_s[:, q4s], x_d.ap()[:, q4s])
            nc.scalar.dma_start(w_s[:], w_d.ap())
            nc.scalar.dma_start(biasf[:], b_d.ap())
            nc.scalar.dma_start(bv_s[:], bv_d.ap())
            for q2 in range(2):
                q2s = slice(q2 * 2048, (q2 + 1) * 2048)
                nc.scalar.dma_start(kz[16:128, q2s], z_d.ap()[0:112, q2s])
            nc.gpsimd.memset(ones16[:], 1.0)
            nc.gpsimd.memset(zerob[:], 0.0)
            # Warm the exp table set while DMAs run.
            nc.scalar.activation(scratch1[:], zerob[:], EXP, bias=zerob[:])

            # qkv projection: psum[0:32] = w[:, 0:32]^T @ x, + bias plane.
            for ch in range(8):
                cs = slice(ch * 512, (ch + 1) * 512)
                ps = stp.tile([32, 512], f32, tag="st")
                nc.tensor.matmul(ps[:], lhsT=w_s[:, 0:32], rhs=x_s[:, cs],
                                 start=True, stop=True)
                nc.vector.tensor_tensor(qz[0:32, cs], ps[:], biasf[:, cs],
                                        ADD)

            # v^T psum staging + vaugT const regions; the projection MMs
            # themselves are interleaved into eighth 0 of the main loop.
            vps = stp.tile([128, 512], f32, tag="st")
            va3 = vaugT[:].rearrange("p (c s) -> p c s", s=33)
            nc.vector.memset(va3[:, :, 16:32], 0.0)
            nc.vector.memset(va3[:, :, 32:33], 1.0)
            vp3 = vps[:].rearrange("p (c s) -> p c s", s=16)

            def emit_vt(jb):
                nc.tensor.matmul(vps[:, jb * 16:(jb + 1) * 16],
                                 lhsT=x_s[:, jb * JW:(jb + 1) * JW],
                                 rhs=w_s[:, 32:48],
                                 start=True, stop=True)

            def emit_vt_copy(c0, c1):
                nc.vector.tensor_copy(va3[:, c0:c1, 0:16], vp3[:, c0:c1, :])

            # k' into the zero-padded stationary tensor (partition shift).
            for q8 in range(8):
                q8s = slice(q8 * 512, (q8 + 1) * 512)
                nc.gpsimd.dma_start(kz[0:DH, q8s], qz[DH:2 * DH, q8s])

            def make_pv(pt, jbs, acc, start, stop):
                def emit():
                    for t, jb in enumerate(jbs):
                        nc.tensor.matmul(
                            acc[:],
                            lhsT=vaugT[:, 33 * jb:33 * jb + 33],
                            rhs=pt[:, 512 * t:512 * (t + 1)],
                            start=(start and t == 0),
                            stop=(stop and t == len(jbs) - 1))
                return emit

            def make_tail_a(acc):
                o17 = op.tile([33, ECOLS], f32, tag="o17")
                r = op.tile([1, ECOLS], f32, tag="r")

                def emit():
                    nc.vector.tensor_copy(o17[:], acc[:])
                    nc.vector.reciprocal(r[:], o17[32:33, :])
                return emit, o17, r

            def make_tail_b(o17, r, es):
                def emit():
                    bc = stp.tile([DH, ECOLS], f32, tag="st")
                    nc.tensor.matmul(bc[:], lhsT=ones16[:], rhs=r[:],
                                     start=True, stop=True)
                    ost = op.tile([DH, ECOLS], f32, tag="ost")
                    nc.vector.tensor_tensor(ost[:], o17[0:DH, :], bc[:],
                                            MULT)
                    nc.vector.tensor_scalar_add(ost[:], ost[:], bv_s[:])
                    nc.sync.dma_start(o_d.ap()[:, es], ost[:])
                return emit

            from collections import deque
            pend = deque()
            GROUPS = [(2 * g, 2 * g + 1) for g in range(NG)]
            pending_a = None
            pending_b = None
            for e in range(NE):
                es = slice(e * ECOLS, (e + 1) * ECOLS)
                acc = accp.tile([33, ECOLS], f32)
                for gi, jbs in enumerate(GROUPS):
                    if e == 0:
                        for jb in jbs:
                            emit_vt(jb)
                        emit_vt_copy(jbs[0], jbs[-1] + 1)
                    fw = 512 * len(jbs)
                    st = stp.tile([128, 1024], f32, tag="st")
                    for t, jb in enumerate(jbs):
                        nc.tensor.matmul(st[:, 512 * t:512 * (t + 1)],
                                         lhsT=kz[:, jb * JW:(jb + 1) * JW],
                                         rhs=qz[:, es],
                                         start=True, stop=True)
                    pt = ptp.tile([128, 1024], bf16)
                    nc.scalar.activation(pt[:, 0:fw], st[:, 0:fw], EXP,
                                         bias=zerob[:])
                    if len(pend) >= 2:
                        pend.popleft()()
                    if pending_a is not None and gi == 2:
                        pending_a()
                        pending_a = None
                    if pending_b is not None and gi == 6:
                        pending_b()
                        pending_b = None
                    pend.append(make_pv(pt, jbs, acc,
                                        start=(gi == 0),
                                        stop=(gi == len(GROUPS) - 1)))
                emit_a, o17, r = make_tail_a(acc)
                pending_a = emit_a
                if pending_b is not None:
                    pending_b()
                pending_b = make_tail_b(o17, r, es)
            while pend:
                pend.popleft()()
            pending_a()
            pending_b()

    nc.compile()
    return nc


def _get_program():
    global _compiled
    if _compiled is None:
        _compiled = _build_program()
    return _compiled


def _prepare_core_inputs(x, w_qkv, b_qkv, emb_d, emb_h, emb_w):
    x2 = np.ascontiguousarray(
        np.asarray(x, np.float32).reshape(C, N)).astype(np.float16)
    w_qkv = np.asarray(w_qkv, np.float32)
    b_qkv = np.asarray(b_qkv, np.float32)
    scale = DH ** -0.5
    emb = (np.asarray(emb_d, np.float32)
           + np.asarray(emb_h, np.float32)
           + np.asarray(emb_w, np.float32)).reshape(DH, N)
    zeros = np.zeros((C, N), np.float16)
    in_maps = []
    for h in range(NHEADS):
        qc = slice(h * DH, (h + 1) * DH)
        kc = slice(DV + h * DH, DV + (h + 1) * DH)
        vc = slice(2 * DV + h * DH, 2 * DV + (h + 1) * DH)
        w = np.empty((C, 48), np.float32)
        w[:, 0:16] = w_qkv[:, qc] * scale
        w[:, 16:32] = w_qkv[:, kc]
        w[:, 32:48] = w_qkv[:, vc]
        w = w.astype(np.float16)
        bias = np.empty((32, N), np.float32)
        bias[0:16, :] = (b_qkv[qc] * scale)[:, None]
        bias[16:32, :] = b_qkv[kc][:, None] + emb
        bv = np.ascontiguousarray(b_qkv[vc][:, None])
        in_maps.append({"x": x2, "w": w, "bias": bias, "bv": bv,
                        "zeros": zeros})
    return in_maps


def kernel(x, w_qkv, b_qkv, emb_d, emb_h, emb_w):
    from concourse.bass_utils import run_bass_kernel_spmd

    nc = _get_program()
    in_maps = _prepare_core_inputs(x, w_qkv, b_qkv, emb_d, emb_h, emb_w)
    res = run_bass_kernel_spmd(nc, in_maps, list(range(NHEADS)))
    out = np.empty((DV, N), np.float32)
    for h in range(NHEADS):
        out[h * DH:(h + 1) * DH, :] = res.results[h]["out"]
    return out.reshape(1, DV, 16, 16, 16)


# revision 20
# speedup vs baseline: 1.0127x; 1.0127x over previous
"""MHSA3D Trainium2 kernel: 8-way head-parallel flash-style attention.

Problem (hardcoded): B=1, C=128, D=H=W=16 -> N=4096 tokens, 8 heads,
dh=16, dv=128.  Each of the 8 NeuronCores computes one head end-to-end:
qkv projection (its head's slice), S^T = k'^T q' logits in [j, i]
layout, exp on ScalarE (no max subtraction -- fp32 exp cannot overflow
for this data), PV accumulation with an appended ones-row producing the
softmax denominator, then normalize.

Perf-critical choices:
- The qk contraction is only dh=16, but K=16 matmuls never trip the PE
  HAM activity monitor, so the PE clock stays throttled at 1.2 GHz.
  We zero-pad the contraction to K=128 (stationary k' rows 16-127 are
  exact zeros); matmul time scales with N only, and full-K matmuls keep
  the PE warm at 2.4 GHz.
- q'/k' in fp16 (logits reach +-46, but q/k values are small; fp16's
  extra mantissa bits halve the logit rounding error vs bf16), P and v
  in bf16 (P reaches exp(40), beyond fp16 range).
- PV for group g is emitted after qk for group g+1 so the PE FIFO never
  head-of-line blocks on the exp; the per-eighth normalize tail is
  deferred into the next eighth the same way.

Host side: fold the 1/sqrt(dh) scale into wq/bq, fold (b_k + positional
embedding) into a [16, N] bias plane, slice per-head weights, run the
SPMD program on cores 0-7, and concatenate the per-head [16, N] outputs.
"""

import numpy as np

NHEADS = 8
DV = 128
DH = DV // NHEADS  # 16
C = 128
N = 4096
ECOLS = 512        # i-columns handled per output tile ("eighth")
NE = N // ECOLS    # 8
JW = 128           # keys per j-block
NJB = N // JW      # 32
NG = NJB // 2      # 16 j-block pairs per eighth

_compiled = None


def _build_program():
    import concourse.bacc as bacc
    import concourse.mybir as mybir
    import concourse.tile as tile

    f32 = mybir.dt.float32
    bf16 = mybir.dt.bfloat16
    fp16 = mybir.dt.float16
    EXP = mybir.ActivationFunctionType.Exp
    ADD = mybir.AluOpType.add
    MULT = mybir.AluOpType.mult

    nc = bacc.Bacc("TRN2", target_bir_lowering=False, debug=False,
                   num_devices=NHEADS)

    x_d = nc.dram_tensor("x", [C, N], fp16, kind="ExternalInput")
    # w cols: 0-15 wq*scale, 16-31 wk, 32-47 wv
    w_d = nc.dram_tensor("w", [C, 48], fp16, kind="ExternalInput")
    # bias plane rows: 0-15 bq*scale (bcast), 16-31 bk+emb
    b_d = nc.dram_tensor("bias", [32, N], f32, kind="ExternalInput")
    bv_d = nc.dram_tensor("bv", [DH, 1], f32, kind="ExternalInput")
    z_d = nc.dram_tensor("zeros", [80, N], fp16, kind="ExternalInput")
    o_d = nc.dram_tensor("out", [DH, N], f32, kind="ExternalOutput")

    with tile.TileContext(nc) as tc:
        with (
            tc.tile_pool(name="const", bufs=1) as const,
            tc.tile_pool(name="pt", bufs=4) as ptp,
            tc.tile_pool(name="o", bufs=2) as op,
            tc.tile_pool(name="st", bufs=3, space="PSUM") as stp,
            tc.tile_pool(name="acc", bufs=2, space="PSUM") as accp,
        ):
            x_s = const.tile([C, N], fp16)
            w_s = const.tile([C, 48], fp16)
            biasf = const.tile([32, N], f32)
            bv_s = const.tile([DH, 1], f32)
            # K=96 contraction: the smallest K that keeps the PE HAM warm.
            # qz rows: 0-15 q'; 16-31 k' (junk for the matmul); 32-95 zero
            qz = const.tile([96, N], fp16)
            # kz rows: 0-15 k'; 16-95 exact zero (masks the qz junk rows)
            kz = const.tile([96, N], fp16)
            vaugT = const.tile([128, 33 * NJB], bf16)  # [j, (v^T |0*16| 1)]
            ones16 = const.tile([1, DH], f32)
            zerob = const.tile([128, 1], f32)
            scratch1 = const.tile([128, 1], f32)

            # Zeros before the bias-adds: Tile tracks WAW at tile
            # granularity, so writes into qz/kz serialize behind zero-fill.
            nc.vector.memset(qz[32:64, :], 0.0)
            nc.vector.memset(qz[64:96, :], 0.0)
            for q4 in range(4):
                q4s = slice(q4 * 1024, (q4 + 1) * 1024)
                nc.sync.dma_start(x_s[:, q4s], x_d.ap()[:, q4s])
            nc.scalar.dma_start(w_s[:], w_d.ap())
            nc.scalar.dma_start(biasf[:], b_d.ap())
            nc.scalar.dma_start(bv_s[:], bv_d.ap())
            for q2 in range(2):
                q2s = slice(q2 * 2048, (q2 + 1) * 2048)
                nc.scalar.dma_start(kz[16:96, q2s], z_d.ap()[0:80, q2s])
            nc.gpsimd.memset(ones16[:], 1.0)
            nc.gpsimd.memset(zerob[:], 0.0)
            # Warm the exp table set while DMAs run.
            nc.scalar.activation(scratch1[:], zerob[:], EXP, bias=zerob[:])

            # qkv projection: psum[0:32] = w[:, 0:32]^T @ x, + bias plane.
            for ch in range(8):
                cs = slice(ch * 512, (ch + 1) * 512)
                ps = stp.tile([32, 512], f32, tag="st")
                nc.tensor.matmul(ps[:], lhsT=w_s[:, 0:32], rhs=x_s[:, cs],
                                 start=True, stop=True)
                nc.vector.tensor_tensor(qz[0:32, cs], ps[:], biasf[:, cs],
                                        ADD)

            # v^T psum staging + vaugT const regions; the projection MMs
            # themselves are interleaved into eighth 0 of the main loop.
            vps = stp.tile([128, 512], f32, tag="st")
            va3 = vaugT[:].rearrange("p (c s) -> p c s", s=33)
            nc.vector.memset(va3[:, :, 16:32], 0.0)
            nc.vector.memset(va3[:, :, 32:33], 1.0)
            vp3 = vps[:].rearrange("p (c s) -> p c s", s=16)

            def emit_vt(jb):
                nc.tensor.matmul(vps[:, jb * 16:(jb + 1) * 16],
                                 lhsT=x_s[:, jb * JW:(jb + 1) * JW],
                                 rhs=w_s[:, 32:48],
                                 start=True, stop=True)

            def emit_vt_copy(c0, c1):
                nc.vector.tensor_copy(va3[:, c0:c1, 0:16], vp3[:, c0:c1, :])

            # k' into the zero-padded stationary tensor (partition shift).
            for q8 in range(8):
                q8s = slice(q8 * 512, (q8 + 1) * 512)
                nc.gpsimd.dma_start(kz[0:DH, q8s], qz[DH:2 * DH, q8s])

            def make_pv(pt, jbs, acc, start, stop):
                def emit():
                    for t, jb in enumerate(jbs):
                        nc.tensor.matmul(
                            acc[:],
                            lhsT=vaugT[:, 33 * jb:33 * jb + 33],
                            rhs=pt[:, 512 * t:512 * (t + 1)],
                            start=(start and t == 0),
                            stop=(stop and t == len(jbs) - 1))
                return emit

            def make_tail_a(acc):
                o17 = op.tile([33, ECOLS], f32, tag="o17")
                r = op.tile([1, ECOLS], f32, tag="r")

                def emit():
                    nc.vector.tensor_copy(o17[:], acc[:])
                    nc.vector.reciprocal(r[:], o17[32:33, :])
                return emit, o17, r

            def make_tail_b(o17, r, es):
                def emit():
                    bc = stp.tile([DH, ECOLS], f32, tag="st")
                    nc.tensor.matmul(bc[:], lhsT=ones16[:], rhs=r[:],
                                     start=True, stop=True)
                    ost = op.tile([DH, ECOLS], f32, tag="ost")
                    nc.vector.tensor_tensor(ost[:], o17[0:DH, :], bc[:],
                                            MULT)
                    nc.vector.tensor_scalar_add(ost[:], ost[:], bv_s[:])
                    nc.sync.dma_start(o_d.ap()[:, es], ost[:])
                return emit

            from collections import deque
            pend = deque()
            GROUPS = [(2 * g, 2 * g + 1) for g in range(NG)]
            pending_a = None
            pending_b = None
            for e in range(NE):
                es = slice(e * ECOLS, (e + 1) * ECOLS)
                acc = accp.tile([33, ECOLS], f32)
                for gi, jbs in enumerate(GROUPS):
                    if e == 0:
                        for jb in jbs:
                            emit_vt(jb)
                        emit_vt_copy(jbs[0], jbs[-1] + 1)
                    fw = 512 * len(jbs)
                    st = stp.tile([128, 1024], f32, tag="st")
                    for t, jb in enumerate(jbs):
                        nc.tensor.matmul(st[:, 512 * t:512 * (t + 1)],
                                         lhsT=kz[:, jb * JW:(jb + 1) * JW],
                                         rhs=qz[:, es],
                                         start=True, stop=True)
                    pt = ptp.tile([128, 1024], bf16)
                    nc.scalar.activation(pt[:, 0:fw], st[:, 0:fw], EXP,
                                         bias=zerob[:])
                    if len(pend) >= 2:
                        pend.popleft()()
                    if pending_a is not None and gi == 2:
                        pending_a()
                        pending_a = None
                    if pending_b is not None and gi == 6:
                        pending_b()
                        pending_b = None
                    pend.append(make_pv(pt, jbs, acc,
                                        start=(gi == 0),
                                        stop=(gi == len(GROUPS) - 1)))
                emit_a, o17, r = make_tail_a(acc)
                pending_a = emit_a
                if pending_b is not None:
                    pending_b()
                pending_b = make_tail_b(o17, r, es)
            while pend:
                pend.popleft()()
            pending_a()
            pending_b()

    nc.compile()
    return nc


def _get_program():
    global _compiled
    if _compiled is None:
        _compiled = _build_program()
    return _compiled


def _prepare_core_inputs(x, w_qkv, b_qkv, emb_d, emb_h, emb_w):
    x2 = np.ascontiguousarray(
        np.asarray(x, np.float32).reshape(C, N)).astype(np.float16)
    w_qkv = np.asarray(w_qkv, np.float32)
    b_qkv = np.asarray(b_qkv, np.float32)
    scale = DH ** -0.5
    emb = (np.asarray(emb_d, np.float32)
           + np.asarray(emb_h, np.float32)
           + np.asarray(emb_w, np.float32)).reshape(DH, N)
    zeros = np.zeros((80, N), np.float16)
    in_maps = []
    for h in range(NHEADS):
        qc = slice(h * DH, (h + 1) * DH)
        kc = slice(DV + h * DH, DV + (h + 1) * DH)
        vc = slice(2 * DV + h * DH, 2 * DV + (h + 1) * DH)
        w = np.empty((C, 48), np.float32)
        w[:, 0:16] = w_qkv[:, qc] * scale
        w[:, 16:32] = w_qkv[:, kc]
        w[:, 32:48] = w_qkv[:, vc]
        w = w.astype(np.float16)
        bias = np.empty((32, N), np.float32)
        bias[0:16, :] = (b_qkv[qc] * scale)[:, None]
        bias[16:32, :] = b_qkv[kc][:, None] + emb
        bv = np.ascontiguousarray(b_qkv[vc][:, None])
        in_maps.append({"x": x2, "w": w, "bias": bias, "bv": bv,
                        "zeros": zeros})
    return in_maps


def kernel(x, w_qkv, b_qkv, emb_d, emb_h, emb_w):
    from concourse.bass_utils import run_bass_kernel_spmd

    nc = _get_program()
    in_maps = _prepare_core_inputs(x, w_qkv, b_qkv, emb_d, emb_h, emb_w)
    res = run_bass_kernel_spmd(nc, in_maps, list(range(NHEADS)))
    out = np.empty((DV, N), np.float32)
    for h in range(NHEADS):
        out[h * DH:(h + 1) * DH, :] = res.results[h]["out"]
    return out.reshape(1, DV, 16, 16, 16)


# revision 22
# speedup vs baseline: 1.0195x; 1.0067x over previous
"""MHSA3D Trainium2 kernel: 8-way head-parallel flash-style attention.

Problem (hardcoded): B=1, C=128, D=H=W=16 -> N=4096 tokens, 8 heads,
dh=16, dv=128.  Each of the 8 NeuronCores computes one head end-to-end:
qkv projection (its head's slice), S^T = k'^T q' logits in [j, i]
layout, exp on ScalarE (no max subtraction -- fp32 exp cannot overflow
for this data), PV accumulation with an appended ones-row producing the
softmax denominator, then normalize.

Perf-critical choices:
- The qk contraction is only dh=16, but K=16 matmuls never trip the PE
  HAM activity monitor, so the PE clock stays throttled at 1.2 GHz.
  We zero-pad the contraction to K=128 (stationary k' rows 16-127 are
  exact zeros); matmul time scales with N only, and full-K matmuls keep
  the PE warm at 2.4 GHz.
- q'/k' in fp16 (logits reach +-46, but q/k values are small; fp16's
  extra mantissa bits halve the logit rounding error vs bf16), P and v
  in bf16 (P reaches exp(40), beyond fp16 range).
- PV for group g is emitted after qk for group g+1 so the PE FIFO never
  head-of-line blocks on the exp; the per-eighth normalize tail is
  deferred into the next eighth the same way.

Host side: fold the 1/sqrt(dh) scale into wq/bq, fold (b_k + positional
embedding) into a [16, N] bias plane, slice per-head weights, run the
SPMD program on cores 0-7, and concatenate the per-head [16, N] outputs.
"""

import numpy as np

NHEADS = 8
DV = 128
DH = DV // NHEADS  # 16
C = 128
N = 4096
ECOLS = 512        # i-columns handled per output tile ("eighth")
NE = N // ECOLS    # 8
JW = 128           # keys per j-block
NJB = N // JW      # 32
NG = NJB // 2      # 16 j-block pairs per eighth

_compiled = None


def _build_program():
    import concourse.bacc as bacc
    import concourse.mybir as mybir
    import concourse.tile as tile

    f32 = mybir.dt.float32
    bf16 = mybir.dt.bfloat16
    fp16 = mybir.dt.float16
    EXP = mybir.ActivationFunctionType.Exp
    ADD = mybir.AluOpType.add
    MULT = mybir.AluOpType.mult

    nc = bacc.Bacc("TRN2", target_bir_lowering=False, debug=False,
                   num_devices=NHEADS)

    x_d = nc.dram_tensor("x", [C, N], fp16, kind="ExternalInput")
    # w cols: 0-15 wq*scale, 16-31 wk, 32-47 wv
    w_d = nc.dram_tensor("w", [C, 48], fp16, kind="ExternalInput")
    # bias plane rows: 0-15 bq*scale (bcast), 16-31 bk+emb
    b_d = nc.dram_tensor("bias", [32, N], f32, kind="ExternalInput")
    bv_d = nc.dram_tensor("bv", [DH, 1], f32, kind="ExternalInput")
    z_d = nc.dram_tensor("zeros", [80, N], fp16, kind="ExternalInput")
    o_d = nc.dram_tensor("out", [DH, N], f32, kind="ExternalOutput")

    with tile.TileContext(nc) as tc:
        with (
            tc.tile_pool(name="const", bufs=1) as const,
            tc.tile_pool(name="pt", bufs=4) as ptp,
            tc.tile_pool(name="o", bufs=2) as op,
            tc.tile_pool(name="st", bufs=3, space="PSUM") as stp,
            tc.tile_pool(name="acc", bufs=2, space="PSUM") as accp,
        ):
            x_s = const.tile([C, N], fp16)
            w_s = const.tile([C, 48], fp16)
            biasf = const.tile([32, N], f32)
            bv_s = const.tile([DH, 1], f32)
            # K=96 contraction: the smallest K that keeps the PE HAM warm.
            # Split into per-512-column tiles so consumers only wait on
            # their own chunk (Tile tracks deps at whole-tile granularity).
            # qz rows: 0-15 q'; 16-31 k' (junk for the matmul); 32-95 zero
            qzt = [const.tile([96, 512], fp16, name=f"qzt{c}")
                   for c in range(8)]
            # kz rows: 0-15 k'; 16-95 exact zero (masks the qz junk rows)
            kzt = [const.tile([96, 512], fp16, name=f"kzt{c}")
                   for c in range(8)]
            vaugT = const.tile([128, 33 * NJB], bf16)  # [j, (v^T |0*16| 1)]
            ones16 = const.tile([1, DH], f32)
            zerob = const.tile([128, 1], f32)
            scratch1 = const.tile([128, 1], f32)

            # Zeros before the bias-adds: Tile tracks WAW at tile
            # granularity, so writes into qz/kz serialize behind zero-fill.
            for t in qzt:
                nc.vector.memset(t[32:64, :], 0.0)
                nc.vector.memset(t[64:96, :], 0.0)
            for q4 in range(4):
                q4s = slice(q4 * 1024, (q4 + 1) * 1024)
                nc.sync.dma_start(x_s[:, q4s], x_d.ap()[:, q4s])
            nc.scalar.dma_start(w_s[:], w_d.ap())
            nc.scalar.dma_start(biasf[:], b_d.ap())
            nc.scalar.dma_start(bv_s[:], bv_d.ap())
            for c in range(8):
                cs = slice(c * 512, (c + 1) * 512)
                nc.scalar.dma_start(kzt[c][16:96, :], z_d.ap()[0:80, cs])
            nc.gpsimd.memset(ones16[:], 1.0)
            nc.gpsimd.memset(zerob[:], 0.0)
            # Warm the exp table set while DMAs run.
            nc.scalar.activation(scratch1[:], zerob[:], EXP, bias=zerob[:])

            # qkv projection: psum[0:32] = w[:, 0:32]^T @ x, + bias plane.
            for ch in range(8):
                cs = slice(ch * 512, (ch + 1) * 512)
                ps = stp.tile([32, 512], f32, tag="st")
                nc.tensor.matmul(ps[:], lhsT=w_s[:, 0:32], rhs=x_s[:, cs],
                                 start=True, stop=True)
                nc.vector.tensor_tensor(qzt[ch][0:32, :], ps[:],
                                        biasf[:, cs], ADD)

            # v^T psum staging + vaugT const regions; the projection MMs
            # themselves are interleaved into eighth 0 of the main loop.
            vps = stp.tile([128, 512], f32, tag="st")
            va3 = vaugT[:].rearrange("p (c s) -> p c s", s=33)
            nc.vector.memset(va3[:, :, 16:32], 0.0)
            nc.vector.memset(va3[:, :, 32:33], 1.0)
            vp3 = vps[:].rearrange("p (c s) -> p c s", s=16)

            def emit_vt(jb):
                nc.tensor.matmul(vps[:, jb * 16:(jb + 1) * 16],
                                 lhsT=x_s[:, jb * JW:(jb + 1) * JW],
                                 rhs=w_s[:, 32:48],
                                 start=True, stop=True)

            def emit_vt_copy(c0, c1):
                nc.vector.tensor_copy(va3[:, c0:c1, 0:16], vp3[:, c0:c1, :])

            # k' into the zero-padded stationary tensor (partition shift).
            for c in range(8):
                nc.gpsimd.dma_start(kzt[c][0:DH, :], qzt[c][DH:2 * DH, :])

            def make_pv(pt, jbs, acc, start, stop):
                def emit():
                    for t, jb in enumerate(jbs):
                        nc.tensor.matmul(
                            acc[:],
                            lhsT=vaugT[:, 33 * jb:33 * jb + 33],
                            rhs=pt[:, 512 * t:512 * (t + 1)],
                            start=(start and t == 0),
                            stop=(stop and t == len(jbs) - 1))
                return emit

            def make_tail_a(acc):
                o17 = op.tile([33, ECOLS], f32, tag="o17")
                r = op.tile([1, ECOLS], f32, tag="r")

                def emit():
                    nc.vector.tensor_copy(o17[:], acc[:])
                    nc.vector.reciprocal_approx_fast(r[:], o17[32:33, :])
                return emit, o17, r

            def make_tail_b(o17, r, es):
                def emit():
                    bc = stp.tile([DH, ECOLS], f32, tag="st")
                    nc.tensor.matmul(bc[:], lhsT=ones16[:], rhs=r[:],
                                     start=True, stop=True)
                    ost = op.tile([DH, ECOLS], f32, tag="ost")
                    nc.vector.tensor_tensor(ost[:], o17[0:DH, :], bc[:],
                                            MULT)
                    nc.vector.tensor_scalar_add(ost[:], ost[:], bv_s[:])
                    nc.sync.dma_start(o_d.ap()[:, es], ost[:])
                return emit

            from collections import deque
            pend = deque()
            GROUPS = [(2 * g, 2 * g + 1) for g in range(NG)]
            pending_a = None
            pending_b = None
            for e in range(NE):
                es = slice(e * ECOLS, (e + 1) * ECOLS)
                acc = accp.tile([33, ECOLS], f32)
                for gi, jbs in enumerate(GROUPS):
                    if e == 0:
                        for jb in jbs:
                            emit_vt(jb)
                        emit_vt_copy(jbs[0], jbs[-1] + 1)
                    fw = 512 * len(jbs)
                    st = stp.tile([128, 1024], f32, tag="st")
                    for t, jb in enumerate(jbs):
                        kc = kzt[jb // 4][:, (jb % 4) * JW:(jb % 4 + 1) * JW]
                        nc.tensor.matmul(st[:, 512 * t:512 * (t + 1)],
                                         lhsT=kc, rhs=qzt[e][:],
                                         start=True, stop=True)
                    pt = ptp.tile([128, 1024], bf16)
                    nc.scalar.activation(pt[:, 0:fw], st[:, 0:fw], EXP,
                                         bias=zerob[:])
                    if len(pend) >= 2:
                        pend.popleft()()
                    if pending_a is not None and gi == 2:
                        pending_a()
                        pending_a = None
                    if pending_b is not None and gi == 6:
                        pending_b()
                        pending_b = None
                    pend.append(make_pv(pt, jbs, acc,
                                        start=(gi == 0),
                                        stop=(gi == len(GROUPS) - 1)))
                emit_a, o17, r = make_tail_a(acc)
                pending_a = emit_a
                if pending_b is not None:
                    pending_b()
                pending_b = make_tail_b(o17, r, es)
            while pend:
                pend.popleft()()
            pending_a()
            pending_b()

    nc.compile()
    return nc


def _get_program():
    global _compiled
    if _compiled is None:
        _compiled = _build_program()
    return _compiled


def _prepare_core_inputs(x, w_qkv, b_qkv, emb_d, emb_h, emb_w):
    x2 = np.ascontiguousarray(
        np.asarray(x, np.float32).reshape(C, N)).astype(np.float16)
    w_qkv = np.asarray(w_qkv, np.float32)
    b_qkv = np.asarray(b_qkv, np.float32)
    scale = DH ** -0.5
    emb = (np.asarray(emb_d, np.float32)
           + np.asarray(emb_h, np.float32)
           + np.asarray(emb_w, np.float32)).reshape(DH, N)
    zeros = np.zeros((80, N), np.float16)
    in_maps = []
    for h in range(NHEADS):
        qc = slice(h * DH, (h + 1) * DH)
        kc = slice(DV + h * DH, DV + (h + 1) * DH)
        vc = slice(2 * DV + h * DH, 2 * DV + (h + 1) * DH)
        w = np.empty((C, 48), np.float32)
        w[:, 0:16] = w_qkv[:, qc] * scale
        w[:, 16:32] = w_qkv[:, kc]
        w[:, 32:48] = w_qkv[:, vc]
        w = w.astype(np.float16)
        bias = np.empty((32, N), np.float32)
        bias[0:16, :] = (b_qkv[qc] * scale)[:, None]
        bias[16:32, :] = b_qkv[kc][:, None] + emb
        bv = np.ascontiguousarray(b_qkv[vc][:, None])
        in_maps.append({"x": x2, "w": w, "bias": bias, "bv": bv,
                        "zeros": zeros})
    return in_maps


def kernel(x, w_qkv, b_qkv, emb_d, emb_h, emb_w):
    from concourse.bass_utils import run_bass_kernel_spmd

    nc = _get_program()
    in_maps = _prepare_core_inputs(x, w_qkv, b_qkv, emb_d, emb_h, emb_w)
    res = run_bass_kernel_spmd(nc, in_maps, list(range(NHEADS)))
    out = np.empty((DV, N), np.float32)
    for h in range(NHEADS):
        out[h * DH:(h + 1) * DH, :] = res.results[h]["out"]
    return out.reshape(1, DV, 16, 16, 16)


# revision 23
# speedup vs baseline: 1.0344x; 1.0147x over previous
"""MHSA3D Trainium2 kernel: 8-way head-parallel flash-style attention.

Problem (hardcoded): B=1, C=128, D=H=W=16 -> N=4096 tokens, 8 heads,
dh=16, dv=128.  Each of the 8 NeuronCores computes one head end-to-end:
qkv projection (its head's slice), S^T = k'^T q' logits in [j, i]
layout, exp on ScalarE (no max subtraction -- fp32 exp cannot overflow
for this data), PV accumulation with an appended ones-row producing the
softmax denominator, then normalize.

Perf-critical choices:
- The qk contraction is only dh=16, but K=16 matmuls never trip the PE
  HAM activity monitor, so the PE clock stays throttled at 1.2 GHz.
  We zero-pad the contraction to K=128 (stationary k' rows 16-127 are
  exact zeros); matmul time scales with N only, and full-K matmuls keep
  the PE warm at 2.4 GHz.
- q'/k' in fp16 (logits reach +-46, but q/k values are small; fp16's
  extra mantissa bits halve the logit rounding error vs bf16), P and v
  in bf16 (P reaches exp(40), beyond fp16 range).
- PV for group g is emitted after qk for group g+1 so the PE FIFO never
  head-of-line blocks on the exp; the per-eighth normalize tail is
  deferred into the next eighth the same way.

Host side: fold the 1/sqrt(dh) scale into wq/bq, fold (b_k + positional
embedding) into a [16, N] bias plane, slice per-head weights, run the
SPMD program on cores 0-7, and concatenate the per-head [16, N] outputs.
"""

import numpy as np

NHEADS = 8
DV = 128
DH = DV // NHEADS  # 16
C = 128
N = 4096
ECOLS = 512        # i-columns handled per output tile ("eighth")
NE = N // ECOLS    # 8
JW = 128           # keys per j-block
NJB = N // JW      # 32
NG = NJB // 2      # 16 j-block pairs per eighth

_compiled = None


def _build_program():
    import concourse.bacc as bacc
    import concourse.mybir as mybir
    import concourse.tile as tile

    f32 = mybir.dt.float32
    bf16 = mybir.dt.bfloat16
    fp16 = mybir.dt.float16
    EXP = mybir.ActivationFunctionType.Exp
    ADD = mybir.AluOpType.add
    MULT = mybir.AluOpType.mult

    nc = bacc.Bacc("TRN2", target_bir_lowering=False, debug=False,
                   num_devices=NHEADS)

    x_d = nc.dram_tensor("x", [C, N], fp16, kind="ExternalInput")
    # w cols: 0-15 wq*scale, 16-31 wk, 32-47 wv
    w_d = nc.dram_tensor("w", [C, 48], fp16, kind="ExternalInput")
    # bias plane rows: 0-15 bq*scale (bcast), 16-31 bk+emb
    b_d = nc.dram_tensor("bias", [32, N], f32, kind="ExternalInput")
    bv_d = nc.dram_tensor("bv", [DH, 1], f32, kind="ExternalInput")
    z_d = nc.dram_tensor("zeros", [80, N], fp16, kind="ExternalInput")
    o_d = nc.dram_tensor("out", [DH, N], f32, kind="ExternalOutput")

    with tile.TileContext(nc) as tc:
        with (
            tc.tile_pool(name="const", bufs=1) as const,
            tc.tile_pool(name="pt", bufs=4) as ptp,
            tc.tile_pool(name="o", bufs=2) as op,
            tc.tile_pool(name="st", bufs=3, space="PSUM") as stp,
            tc.tile_pool(name="acc", bufs=2, space="PSUM") as accp,
        ):
            x_s = const.tile([C, N], fp16)
            w_s = const.tile([C, 48], fp16)
            biasf = const.tile([32, N], f32)
            bv_s = const.tile([DH, 1], f32)
            # K=96 contraction: the smallest K that keeps the PE HAM warm.
            # Split into per-512-column tiles so consumers only wait on
            # their own chunk (Tile tracks deps at whole-tile granularity).
            # qz rows: 0-15 q'; 16-31 k' (junk for the matmul); 32-95 zero
            qzt = [const.tile([96, 512], fp16, name=f"qzt{c}")
                   for c in range(8)]
            # kz rows: 0-15 k'; 16-95 exact zero (masks the qz junk rows)
            kzt = [const.tile([96, 512], fp16, name=f"kzt{c}")
                   for c in range(8)]
            vaugT = const.tile([128, 33 * NJB], bf16)  # [j, (v^T |0*16| 1)]
            ones16 = const.tile([1, DH], f32)
            zerob = const.tile([128, 1], f32)
            scratch1 = const.tile([128, 1], f32)

            # Zeros before the bias-adds: Tile tracks WAW at tile
            # granularity, so writes into qz/kz serialize behind zero-fill.
            for c in range(8):
                nc.gpsimd.dma_start(qzt[c][32:96, :],
                                    z_d.ap()[0:64, c * 512:(c + 1) * 512])
            for q4 in range(4):
                q4s = slice(q4 * 1024, (q4 + 1) * 1024)
                nc.sync.dma_start(x_s[:, q4s], x_d.ap()[:, q4s])
            nc.scalar.dma_start(w_s[:], w_d.ap())
            nc.scalar.dma_start(biasf[:], b_d.ap())
            nc.scalar.dma_start(bv_s[:], bv_d.ap())
            for c in range(8):
                cs = slice(c * 512, (c + 1) * 512)
                nc.scalar.dma_start(kzt[c][16:96, :], z_d.ap()[0:80, cs])
            nc.gpsimd.memset(ones16[:], 1.0)
            nc.gpsimd.memset(zerob[:], 0.0)
            # Warm the exp table set while DMAs run.
            nc.scalar.activation(scratch1[:], zerob[:], EXP, bias=zerob[:])

            # qkv projection: psum[0:32] = w[:, 0:32]^T @ x, + bias plane.
            for ch in range(8):
                cs = slice(ch * 512, (ch + 1) * 512)
                ps = stp.tile([32, 512], f32, tag="st")
                nc.tensor.matmul(ps[:], lhsT=w_s[:, 0:32], rhs=x_s[:, cs],
                                 start=True, stop=True)
                nc.vector.tensor_tensor(qzt[ch][0:32, :], ps[:],
                                        biasf[:, cs], ADD)

            # v^T psum staging + vaugT const regions; the projection MMs
            # themselves are interleaved into eighth 0 of the main loop.
            vps = stp.tile([128, 512], f32, tag="st")
            va3 = vaugT[:].rearrange("p (c s) -> p c s", s=33)
            nc.vector.memset(va3[:, :, 16:32], 0.0)
            nc.vector.memset(va3[:, :, 32:33], 1.0)
            vp3 = vps[:].rearrange("p (c s) -> p c s", s=16)

            def emit_vt(jb):
                nc.tensor.matmul(vps[:, jb * 16:(jb + 1) * 16],
                                 lhsT=x_s[:, jb * JW:(jb + 1) * JW],
                                 rhs=w_s[:, 32:48],
                                 start=True, stop=True)

            def emit_vt_copy(c0, c1):
                nc.vector.tensor_copy(va3[:, c0:c1, 0:16], vp3[:, c0:c1, :])

            # k' into the zero-padded stationary tensor (partition shift).
            for c in range(8):
                nc.gpsimd.dma_start(kzt[c][0:DH, :], qzt[c][DH:2 * DH, :])

            def make_pv(pt, jbs, acc, start, stop):
                def emit():
                    for t, jb in enumerate(jbs):
                        nc.tensor.matmul(
                            acc[:],
                            lhsT=vaugT[:, 33 * jb:33 * jb + 33],
                            rhs=pt[:, 512 * t:512 * (t + 1)],
                            start=(start and t == 0),
                            stop=(stop and t == len(jbs) - 1))
                return emit

            def make_tail_a(acc):
                o17 = op.tile([33, ECOLS], f32, tag="o17")
                r = op.tile([1, ECOLS], f32, tag="r")

                def emit():
                    nc.vector.tensor_copy(o17[:], acc[:])
                    nc.vector.reciprocal_approx_fast(r[:], o17[32:33, :])
                return emit, o17, r

            def make_tail_b(o17, r, es):
                def emit():
                    bc = stp.tile([DH, ECOLS], f32, tag="st")
                    nc.tensor.matmul(bc[:], lhsT=ones16[:], rhs=r[:],
                                     start=True, stop=True)
                    ost = op.tile([DH, ECOLS], f32, tag="ost")
                    nc.vector.tensor_tensor(ost[:], o17[0:DH, :], bc[:],
                                            MULT)
                    nc.vector.tensor_scalar_add(ost[:], ost[:], bv_s[:])
                    nc.sync.dma_start(o_d.ap()[:, es], ost[:])
                return emit

            from collections import deque
            pend = deque()
            GROUPS = [(2 * g, 2 * g + 1) for g in range(NG)]
            pending_a = None
            pending_b = None
            for e in range(NE):
                es = slice(e * ECOLS, (e + 1) * ECOLS)
                acc = accp.tile([33, ECOLS], f32)
                for gi, jbs in enumerate(GROUPS):
                    if e == 0:
                        for jb in jbs:
                            emit_vt(jb)
                        emit_vt_copy(jbs[0], jbs[-1] + 1)
                    fw = 512 * len(jbs)
                    st = stp.tile([128, 1024], f32, tag="st")
                    for t, jb in enumerate(jbs):
                        kc = kzt[jb // 4][:, (jb % 4) * JW:(jb % 4 + 1) * JW]
                        nc.tensor.matmul(st[:, 512 * t:512 * (t + 1)],
                                         lhsT=kc, rhs=qzt[e][:],
                                         start=True, stop=True)
                    pt = ptp.tile([128, 1024], bf16)
                    nc.scalar.activation(pt[:, 0:fw], st[:, 0:fw], EXP,
                                         bias=zerob[:])
                    if len(pend) >= 2:
                        pend.popleft()()
                    if pending_a is not None and gi == 2:
                        pending_a()
                        pending_a = None
                    if pending_b is not None and gi == 6:
                        pending_b()
                        pending_b = None
                    pend.append(make_pv(pt, jbs, acc,
                                        start=(gi == 0),
                                        stop=(gi == len(GROUPS) - 1)))
                emit_a, o17, r = make_tail_a(acc)
                pending_a = emit_a
                if pending_b is not None:
                    pending_b()
                pending_b = make_tail_b(o17, r, es)
            while pend:
                pend.popleft()()
            pending_a()
            pending_b()

    nc.compile()
    return nc


def _get_program():
    global _compiled
    if _compiled is None:
        _compiled = _build_program()
    return _compiled


def _prepare_core_inputs(x, w_qkv, b_qkv, emb_d, emb_h, emb_w):
    x2 = np.ascontiguousarray(
        np.asarray(x, np.float32).reshape(C, N)).astype(np.float16)
    w_qkv = np.asarray(w_qkv, np.float32)
    b_qkv = np.asarray(b_qkv, np.float32)
    scale = DH ** -0.5
    emb = (np.asarray(emb_d, np.float32)
           + np.asarray(emb_h, np.float32)
           + np.asarray(emb_w, np.float32)).reshape(DH, N)
    zeros = np.zeros((80, N), np.float16)
    in_maps = []
    for h in range(NHEADS):
        qc = slice(h * DH, (h + 1) * DH)
        kc = slice(DV + h * DH, DV + (h + 1) * DH)
        vc = slice(2 * DV + h * DH, 2 * DV + (h + 1) * DH)
        w = np.empty((C, 48), np.float32)
        w[:, 0:16] = w_qkv[:, qc] * scale
        w[:, 16:32] = w_qkv[:, kc]
        w[:, 32:48] = w_qkv[:, vc]
        w = w.astype(np.float16)
        bias = np.empty((32, N), np.float32)
        bias[0:16, :] = (b_qkv[qc] * scale)[:, None]
        bias[16:32, :] = b_qkv[kc][:, None] + emb
        bv = np.ascontiguousarray(b_qkv[vc][:, None])
        in_maps.append({"x": x2, "w": w, "bias": bias, "bv": bv,
                        "zeros": zeros})
    return in_maps


def kernel(x, w_qkv, b_qkv, emb_d, emb_h, emb_w):
    from concourse.bass_utils import run_bass_kernel_spmd

    nc = _get_program()
    in_maps = _prepare_core_inputs(x, w_qkv, b_qkv, emb_d, emb_h, emb_w)
    res = run_bass_kernel_spmd(nc, in_maps, list(range(NHEADS)))
    out = np.empty((DV, N), np.float32)
    for h in range(NHEADS):
        out[h * DH:(h + 1) * DH, :] = res.results[h]["out"]
    return out.reshape(1, DV, 16, 16, 16)
